# revision 43
# baseline (speedup 1.0000x reference)
"""MixConv depthwise conv (3x3/5x5/7x7 over 64-channel groups) on 8 NeuronCores.

Per core: 24 channels (8 of each kernel size). The 5x5/7x7 channels (and any
k=3 channels not offloaded) run as banded-Toeplitz matmuls on the TensorEngine:
a kxk depthwise conv = sum over dx of a 1D conv along H (a banded [H, H]
Toeplitz matmul contracting over H=112 partitions, folding all k dy-taps),
with W-shifts as free-dim offsets into a padded SBUF tile and dx-passes
accumulating in PSUM (pass-major across all 8 banks).

7 of the 8 3x3 channels are offloaded to the otherwise-idle VectorEngine:
layout [128 partitions = (image, 28-row H-block)], free dim = halo'd 30x114
patch, so all 9 taps are free-dim offsets (engine APs must start at partition
0, so H-shifts cannot be partition offsets). Each tap = tensor_scalar_mul +
tensor_add, ping-ponging two bf16 accumulators; a column-shifted copy of x
keeps every read 4-byte aligned for the DVE 2x/4x packed modes.

Per-core engine balance (cost-model us, validated by isolation timing):
PE ~150, DVE ~150 (7 ch x 21.4), ACT ~78 (all 136 PSUM drains — DSPLIT=0
beat the old 1-DVE/7-ACT split by ~14us measured), DMA ~133 aggregate.
The PE moving operand is a 3D AP [p, 4 img, 112] that SKIPS the inter-image
pad gaps (F=448 instead of 472, PE_PACK="skip": -3.8us measured, and PSUM
images land dense so drains are fully-2D). Output DMAs stay on the SP HWDGE
queue (the ACT-queue variant measured ~8us slower: DMA-trigger waits block
the strict ACT FIFO). Chunk-major PE order (drain banks early) measured
+7us: lhsT then reloads every matmul instead of every 8.

In-situ per-op costs match the rust cost model (DVE TS-mul 0.88us, TT-add
1.69us at [128,3136] bf16; ACT drain 0.56us) — NOT the 2-3x faster numbers
an unloaded microbenchmark reports; k=5 channels on DVE (28u vs 7.9us PE)
and ACT tap-muls were tried and lose end-to-end.

Everything rides in bf16 (PSUM accumulates fp32); HBM traffic halves.
Measured ~160us in-process vs ~173us for the prior best config under
identical conditions (absolute numbers drift +-15% between sessions).
"""

import numpy as np
import ml_dtypes

import concourse.bacc as bacc
import concourse.mybir as mybir
import concourse.tile as tile
from concourse.bass_utils import run_bass_kernel_spmd

BF16 = ml_dtypes.bfloat16

# Problem constants (hardcoded per contract)
N_IMGS = 32
H = W = 112
GROUP_KS = (3, 5, 7)
GROUP_SIZE = 64          # channels per group
N_CORES = 8
CH_PER_GROUP_PER_CORE = GROUP_SIZE // N_CORES   # 8
CH_PER_CORE = CH_PER_GROUP_PER_CORE * len(GROUP_KS)  # 24

# per-image region width in the padded tile: W + 2*pad(k) (gap-trimmed per k)
RW = {7: W + 6, 5: W + 4, 3: W + 2}
XCOLS_K = {k: N_IMGS * RW[k] + 8 for k in RW}  # +8 slack for over-read
XCOLS = XCOLS_K[7]       # staging tensor width (max)
OCOLS = N_IMGS * W
N_MM_K = {k: 4 * RW[k] for k in RW}  # matmul free dim (4 images/chunk)
N_CHUNK = N_IMGS // 4    # 8 chunks = 8 PSUM banks

# DVE-offload layout: partitions = (image, H-block of 28 rows), free = patch
HB = 28
RH = HB + 2              # stored rows per partition (1 halo row each side)
RWP = W + 2              # stored cols per row (1 pad col each side)
XDF = RH * RWP + 4       # 3424 (+4 slack for shifted over-read)
ODF = HB * W             # 3136 out elems per partition
TAPS3 = [(dy, dx) for dy in (-1, 0, 1) for dx in (-1, 0, 1)]

# Tuning (fixed by config sweeps on hardware)
N3 = 7                   # 3x3 channels offloaded to the Vector engine
TAP_MODE = "vs"          # conv taps as tensor_scalar_mul + tensor_add, all DVE
DSPLIT = 0.0             # PSUM banks per channel drained on DVE (rest on ACT)
OUT_Q = "sp"             # DMA queue for outputs: "act" (scalar HWDGE) or "sp"
PE_PACK = "skip"         # "skip": 3D rhs AP skips gap cols (F=448, dense PSUM);
                         # "stream": 2D rhs streams gaps (F=4*rw)
PARITY_TAPS = True       # even-offset taps first (x_o DMA cover)
TAIL_MODE = False        # True: last PE channel drains per-bank — shorter
                         # one-shot tail but ~4us slower in steady state
                         # (per-bank y DMAs + lhsT reload every matmul)
XPOOL_BUFS = 4           # x staging double-buffer depth (PE)
DPOOL_BUFS = 2           # x staging depth (DVE)

MM_MODE = f"bf16 n3={N3} {TAP_MODE}"  # informational (test.py prints it)

# Per-core channel order: interleave 7,5,3 so early big-k channels cover the
# DMA prefetch of later ones.
CORE_KS = [7, 5, 3] * CH_PER_GROUP_PER_CORE
DVE_POS = [i for i, k in enumerate(CORE_KS) if k == 3][:N3]
PE_POS = [i for i in range(CH_PER_CORE) if i not in DVE_POS]
KS_PE = [CORE_KS[i] for i in PE_POS]
TOFF = np.cumsum([0] + KS_PE).tolist()    # tmat slice offset per PE channel
N_TMAT = TOFF[-1]
N_PE = len(PE_POS)

_BASS_CACHE = {}


def _build_bass(reps=1):
    bf = mybir.dt.bfloat16
    nc = bacc.Bacc("TRN2", target_bir_lowering=False, debug=False)
    xp_d = nc.dram_tensor("xp", [N_PE, H, XCOLS], bf, kind="ExternalInput")
    t_d = nc.dram_tensor("tmat", [N_TMAT * H, H], bf, kind="ExternalInput")
    y_d = nc.dram_tensor("y", [N_PE, H, OCOLS], bf, kind="ExternalOutput")
    if N3:
        xd_d = nc.dram_tensor("xd", [N3, 128, XDF], bf, kind="ExternalInput")
        w_d = nc.dram_tensor("wdve", [128, N3 * 9], mybir.dt.float32, kind="ExternalInput")
        y2_d = nc.dram_tensor("y2", [N3, 128, ODF], bf, kind="ExternalOutput")

    with tile.TileContext(nc) as tc:
        with (
            tc.tile_pool(name="xpool", bufs=XPOOL_BUFS) as xpool,
            tc.tile_pool(name="tpool", bufs=1) as tpool,
            tc.tile_pool(name="opool", bufs=3) as opool,
            tc.tile_pool(name="dpool", bufs=DPOOL_BUFS) as dpool,
            tc.tile_pool(name="spool", bufs=3) as spool,
            tc.tile_pool(name="pspool", bufs=8, space="PSUM") as pspool,
        ):
            # Toeplitz bank resident in SBUF, one per-channel slice DMA each.
            t_t = tpool.tile([H, N_TMAT * H], bf, tag="t", name="tmat_sb")
            for ci in range(N_PE):
                k = KS_PE[ci]
                nc.sync.dma_start(
                    t_t[:, TOFF[ci] * H : (TOFF[ci] + k) * H],
                    t_d[TOFF[ci] * H : (TOFF[ci] + k) * H].rearrange(
                        "(p d) m -> p (d m)", d=k
                    ),
                )
            if N3:
                w_t = tpool.tile([128, N3 * 9], mybir.dt.float32, tag="w", name="wdve_sb")
                nc.sync.dma_start(w_t[:, :], w_d[:, :])

            def out_dma(dst, srcap):
                (nc.scalar if OUT_Q == "act" else nc.sync).dma_start(dst, srcap)

            def pe_channel(ci, tail=False):
                k = KS_PE[ci]
                rw, n_mm, xc = RW[k], N_MM_K[k], XCOLS_K[k]
                x_t = xpool.tile([H, XCOLS], bf, tag="x", name=f"x{ci}")
                if ci == 0:
                    # split the critical-path first DMA so pass 0 can start
                    # after half the transfer has landed
                    nc.sync.dma_start(x_t[:, : xc // 2], xp_d[ci][:, : xc // 2])
                    nc.sync.dma_start(x_t[:, xc // 2 : xc], xp_d[ci][:, xc // 2 : xc])
                else:
                    nc.sync.dma_start(x_t[:, :xc], xp_d[ci][:, :xc])
                out_t = opool.tile([H, OCOLS], bf, tag="o", name=f"o{ci}")
                # 8 single-bank PSUM tiles: cross-bank PSUM reads cost a flat
                # ~2.9us, single-bank DVE drains ~0.5us.
                pts = [
                    pspool.tile([H, N_MM_K[7]], mybir.dt.float32, tag="ps",
                                name=f"ps{ci}_{b}")
                    for b in range(N_CHUNK)
                ]

                def mm(dx, b):
                    lhsT = t_t[:, (TOFF[ci] + dx) * H : (TOFF[ci] + dx + 1) * H]
                    base = 4 * b * rw
                    if PE_PACK == "skip":
                        # 3D moving AP streams only the 4x112 data cols,
                        # skipping the pad gaps -> F=448, dense PSUM
                        nc.tensor.matmul(
                            pts[b][:, : 4 * W],
                            lhsT=lhsT,
                            rhs=x_t[:, base + dx : base + dx + n_mm]
                            .rearrange("p (i g) -> p i g", i=4)[:, :, :W],
                            start=(dx == 0),
                            stop=(dx == k - 1),
                        )
                    else:
                        nc.tensor.matmul(
                            pts[b][:, :n_mm],
                            lhsT=lhsT,
                            rhs=x_t[:, base + dx : base + dx + n_mm],
                            start=(dx == 0),
                            stop=(dx == k - 1),
                        )

                ov = out_t.rearrange("p (i w) -> p i w", i=N_IMGS)

                def drain(b):
                    img0 = 4 * b
                    if PE_PACK == "skip":
                        src = pts[b][:, : 4 * W]   # dense: fully-2D drain
                        dst = out_t[:, img0 * W : (img0 + 4) * W]
                    else:
                        src = pts[b][:, :n_mm].rearrange(
                            "p (i r) -> p i r", i=4)[:, :, :W]
                        dst = ov[:, img0 : img0 + 4, :]
                    if b < int(DSPLIT) + (1 if (DSPLIT % 1) and ci % 2 else 0):
                        nc.vector.tensor_copy(out=dst, in_=src)
                    else:
                        nc.scalar.copy(dst, src)
                    if tail:
                        # tail channel: DMA each 4-image piece right away
                        out_dma(y_d[ci][:, img0 * W : (img0 + 4) * W],
                                out_t[:, img0 * W : (img0 + 4) * W])
                    elif b == N_CHUNK // 2 - 1:
                        out_dma(y_d[ci][:, : OCOLS // 2],
                                out_t[:, : OCOLS // 2])

                if tail:
                    # last channel: finish each bank then drain+DMA it, so the
                    # drain/DMA tail overlaps this channel's own matmuls
                    # (lhsT reloads every MM, but only k*8 small MMs here)
                    for b in range(N_CHUNK):
                        for dx in range(k):
                            mm(dx, b)
                        drain(b)
                else:
                    # pass-major: one lhsT load per dx, 8 banks stream under it
                    for dx in range(k):
                        for b in range(N_CHUNK):
                            mm(dx, b)
                    for b in range(N_CHUNK):
                        drain(b)
                    out_dma(y_d[ci][:, OCOLS // 2 :], out_t[:, OCOLS // 2 :])

            def dve_channel(di):
                x_e = dpool.tile([128, XDF], bf, tag="xe", name=f"xe{di}")
                nc.sync.dma_start(x_e[:, :], xd_d[di])
                x_o = dpool.tile([128, XDF], bf, tag="xo", name=f"xo{di}")
                nc.sync.dma_start(x_o[:, : XDF - 1], xd_d[di][:, 1:XDF])
                accs = [
                    spool.tile([128, ODF], bf, tag=f"a{j}", name=f"acc{j}_{di}")
                    for j in range(2)
                ]

                def tap_ap(dy, dx):
                    off = (1 + dy) * RWP + (1 + dx)
                    src, o = (x_e, off) if off % 2 == 0 else (x_o, off - 1)
                    return src[:, o : o + HB * RWP].rearrange(
                        "p (r c) -> p r c", r=HB
                    )[:, :, :W]

                def wap(t):
                    return w_t[:, di * 9 + t : di * 9 + t + 1]

                def scale_to(dst, t):
                    dy, dx = TAPS3[t]
                    # "mix": one scale per channel on DVE, rest on ACT
                    # "vs1": only the last tap's mul on ACT (small DVE relief)
                    on_act = (TAP_MODE == "as" or (TAP_MODE == "mix" and t != 0)
                              or (TAP_MODE == "vs1" and t == 8))
                    if on_act:
                        nc.scalar.activation(
                            dst, tap_ap(dy, dx),
                            mybir.ActivationFunctionType.Copy, scale=wap(t),
                        )
                    else:
                        nc.vector.tensor_scalar_mul(dst, tap_ap(dy, dx), wap(t))

                a3 = [a.rearrange("p (r c) -> p r c", r=HB) for a in accs]
                # even-offset taps (dx=+-1) first: they read x_e only, giving
                # the x_o DMA ~15us of cover before any odd tap needs it
                if PARITY_TAPS:
                    order = sorted(range(9), key=lambda t: (1 + TAPS3[t][1]) % 2)
                else:
                    order = list(range(9))
                scale_to(a3[0], order[0])
                cur = 0
                for pos, t in enumerate(order[1:], start=1):
                    nxt = 1 - cur
                    if TAP_MODE == "stt":
                        dy, dx = TAPS3[t]
                        nc.vector.scalar_tensor_tensor(
                            out=a3[nxt], in0=tap_ap(dy, dx), scalar=wap(t),
                            in1=a3[cur],
                            op0=mybir.AluOpType.mult, op1=mybir.AluOpType.add,
                        )
                    else:
                        s_t = spool.tile([128, ODF], bf, tag="s",
                                         name=f"s{di}_{t}")
                        s3 = s_t.rearrange("p (r c) -> p r c", r=HB)
                        scale_to(s3, t)
                        # "mix": spill 3 of the 8 adds per channel to GPSIMD
                        # "vsg2": spill 2 (Pool idle since drains moved to ACT)
                        spill = ((TAP_MODE == "mix" and t in (2, 5, 8))
                                 or (TAP_MODE == "vsg2" and pos in (3, 6)))
                        if spill:
                            nc.gpsimd.tensor_add(a3[nxt], a3[cur], s3)
                        else:
                            nc.vector.tensor_add(a3[nxt], a3[cur], s3)
                    cur = nxt
                out_dma(y2_d[di], accs[cur][:, :])

            def body():
                pe_i = dve_i = 0
                for pos in range(CH_PER_CORE):
                    if pos in DVE_POS:
                        dve_channel(dve_i)
                        dve_i += 1
                    else:
                        pe_channel(pe_i, tail=TAIL_MODE and pe_i == N_PE - 1)
                        pe_i += 1

            if reps == 1:
                body()
            else:
                with tc.For_i(0, reps, 1):
                    body()
    nc.compile()
    return nc


def _get_bass(reps=1):
    if reps not in _BASS_CACHE:
        _BASS_CACHE[reps] = _build_bass(reps)
    return _BASS_CACHE[reps]


def _build_toeplitz(w, k):
    """w: [C, 1, k, k] -> T: [C, k, H, H], T[c,dx,hin,hout] = w[c,0,hin-hout+pad,dx]."""
    pad = (k - 1) // 2
    C = w.shape[0]
    T = np.zeros((C, k, H, H), np.float32)
    for dy in range(k):
        off = pad - dy  # hout = hin + off
        hin = np.arange(max(0, -off), H - max(0, off))
        T[:, :, hin, hin + off] = w[:, 0, dy, :][:, :, None]
    return T


def _core_channels(core):
    """Global channel ids in this core's processing order (7,5,3 interleave)."""
    out = []
    for j in range(CH_PER_GROUP_PER_CORE):
        for g in (2, 1, 0):  # k=7, 5, 3 groups
            out.append(g * GROUP_SIZE + core * CH_PER_GROUP_PER_CORE + j)
    return out


def _prepare_in_maps(x, w3, w5, w7):
    x = np.asarray(x, dtype=np.float32).astype(BF16)
    ws = {3: np.asarray(w3, np.float32), 5: np.asarray(w5, np.float32),
          7: np.asarray(w7, np.float32)}
    Ts = {k: _build_toeplitz(ws[k], k) for k in (5, 7) if True}
    Ts[3] = _build_toeplitz(ws[3], 3)

    in_maps = []
    for core in range(N_CORES):
        chs = _core_channels(core)
        pe_chs = [chs[i] for i in PE_POS]
        dve_chs = [chs[i] for i in DVE_POS]

        # staged x (PE): [N_PE, H, XCOLS], data at [i*rw+pad, i*rw+pad+W)
        xp = np.zeros((N_PE, H, XCOLS), BF16)
        for ci, gch in enumerate(pe_chs):
            k = KS_PE[ci]
            rw, pad = RW[k], (k - 1) // 2
            xv = xp[ci, :, : N_IMGS * rw].reshape(H, N_IMGS, rw)
            xv[:, :, pad : pad + W] = x[:, gch].transpose(1, 0, 2)

        # tmat blocks: per PE channel [hin, dx, hout] -> [(hin dx), hout]
        blocks = []
        for ci, gch in enumerate(pe_chs):
            k = KS_PE[ci]
            Tc = Ts[k][gch % GROUP_SIZE]  # [dx, hin, hout]
            blocks.append(
                np.ascontiguousarray(Tc.transpose(1, 0, 2)).reshape(k * H, H)
            )
        tml = np.concatenate(blocks, axis=0)
        assert tml.shape[0] == N_TMAT * H
        m = {"xp": xp, "tmat": tml.astype(BF16)}

        if N3:
            # staged x (DVE): [N3, 128, XDF]; partition = img*4 + hblock,
            # free = halo'd 30x114 patch
            xd = np.zeros((N3, 128, XDF), BF16)
            xdv = xd[:, :, : RH * RWP].reshape(N3, N_IMGS, 4, RH, RWP)
            for di, gch in enumerate(dve_chs):
                pad_img = np.zeros((N_IMGS, H + 2, RWP), BF16)
                pad_img[:, 1 : H + 1, 1 : W + 1] = x[:, gch]
                for hb in range(4):
                    xdv[di, :, hb] = pad_img[:, HB * hb : HB * hb + RH, :]
            # tap weights broadcast across partitions: [128, N3*9]
            wd = np.zeros((N3, 9), np.float32)
            for di, gch in enumerate(dve_chs):
                wd[di] = ws[3][gch % GROUP_SIZE, 0].reshape(9)
            m["xd"] = xd
            m["wdve"] = np.ascontiguousarray(
                np.broadcast_to(wd.reshape(1, N3 * 9), (128, N3 * 9))
            )
        in_maps.append(m)
    return in_maps


def _gather(results):
    out = np.empty((N_IMGS, GROUP_SIZE * len(GROUP_KS), H, W), np.float32)
    for core in range(N_CORES):
        chs = _core_channels(core)
        pe_chs = [chs[i] for i in PE_POS]
        y = np.asarray(results[core]["y"]).astype(np.float32)
        y = y.reshape(N_PE, H, N_IMGS, W)
        out[:, pe_chs] = y.transpose(2, 0, 1, 3)
        if N3:
            dve_chs = [chs[i] for i in DVE_POS]
            y2 = np.asarray(results[core]["y2"]).astype(np.float32)
            y2 = y2.reshape(N3, N_IMGS, 4, HB, W)
            for di, gch in enumerate(dve_chs):
                out[:, gch] = y2[di].reshape(N_IMGS, H, W)
    return out


def run(x, w3, w5, w7, **spmd_kwargs):
    """Full run; returns (output, BassKernelResults) for profiling access."""
    nc = _get_bass()
    in_maps = _prepare_in_maps(x, w3, w5, w7)
    br = run_bass_kernel_spmd(nc, in_maps, core_ids=list(range(N_CORES)), **spmd_kwargs)
    return _gather(br.results), br


def kernel(x, w3, w5, w7):
    out, _ = run(x, w3, w5, w7)
    return out



# revision 44
# speedup vs baseline: 1.0529x; 1.0529x over previous
"""MixConv depthwise conv (3x3/5x5/7x7 over 64-channel groups) on 8 NeuronCores.

Per core: 24 channels (8 of each kernel size). The 5x5/7x7 channels (and any
k=3 channels not offloaded) run as banded-Toeplitz matmuls on the TensorEngine:
a kxk depthwise conv = sum over dx of a 1D conv along H (a banded [H, H]
Toeplitz matmul contracting over H=112 partitions, folding all k dy-taps),
with W-shifts as free-dim offsets into a padded SBUF tile and dx-passes
accumulating in PSUM (pass-major across all 8 banks).

7 of the 8 3x3 channels are offloaded to the otherwise-idle VectorEngine:
layout [128 partitions = (image, 28-row H-block)], free dim = halo'd 30x114
patch, so all 9 taps are free-dim offsets (engine APs must start at partition
0, so H-shifts cannot be partition offsets). Each tap = tensor_scalar_mul +
tensor_add, ping-ponging two bf16 accumulators; a column-shifted copy of x
keeps every read 4-byte aligned for the DVE 2x/4x packed modes.

Per-core engine balance (cost-model us, validated by isolation timing):
PE ~150, DVE ~150 (7 ch x 21.4), ACT ~78 (all 136 PSUM drains — DSPLIT=0
beat the old 1-DVE/7-ACT split by ~14us measured), DMA ~133 aggregate.
The PE moving operand is a 3D AP [p, 4 img, 112] that SKIPS the inter-image
pad gaps (F=448 instead of 472, PE_PACK="skip": -3.8us measured, and PSUM
images land dense so drains are fully-2D). Output DMAs stay on the SP HWDGE
queue (the ACT-queue variant measured ~8us slower: DMA-trigger waits block
the strict ACT FIFO). Chunk-major PE order (drain banks early) measured
+7us: lhsT then reloads every matmul instead of every 8.

In-situ per-op costs match the rust cost model (DVE TS-mul 0.88us, TT-add
1.69us at [128,3136] bf16; ACT drain 0.56us) — NOT the 2-3x faster numbers
an unloaded microbenchmark reports; k=5 channels on DVE (28u vs 7.9us PE)
and ACT tap-muls were tried and lose end-to-end.

Everything rides in bf16 (PSUM accumulates fp32); HBM traffic halves.
Measured ~160us in-process vs ~173us for the prior best config under
identical conditions (absolute numbers drift +-15% between sessions).
"""

import numpy as np
import ml_dtypes

import concourse.bacc as bacc
import concourse.mybir as mybir
import concourse.tile as tile
from concourse.bass_utils import run_bass_kernel_spmd

BF16 = ml_dtypes.bfloat16

# Problem constants (hardcoded per contract)
N_IMGS = 32
H = W = 112
GROUP_KS = (3, 5, 7)
GROUP_SIZE = 64          # channels per group
N_CORES = 8
CH_PER_GROUP_PER_CORE = GROUP_SIZE // N_CORES   # 8
CH_PER_CORE = CH_PER_GROUP_PER_CORE * len(GROUP_KS)  # 24

# per-image region width in the padded tile: W + 2*pad(k) (gap-trimmed per k)
RW = {7: W + 6, 5: W + 4, 3: W + 2}
XCOLS_K = {k: N_IMGS * RW[k] + 8 for k in RW}  # +8 slack for over-read
XCOLS = XCOLS_K[7]       # staging tensor width (max)
OCOLS = N_IMGS * W
N_MM_K = {k: 4 * RW[k] for k in RW}  # matmul free dim (4 images/chunk)
N_CHUNK = N_IMGS // 4    # 8 chunks = 8 PSUM banks

# DVE-offload layout: partitions = (image, H-block of 28 rows), free = patch
HB = 28
RH = HB + 2              # stored rows per partition (1 halo row each side)
RWP = W + 2              # stored cols per row (1 pad col each side)
XDF = RH * RWP + 4       # 3424 (+4 slack for shifted over-read)
ODF = HB * W             # 3136 out elems per partition
TAPS3 = [(dy, dx) for dy in (-1, 0, 1) for dx in (-1, 0, 1)]

# Tuning (fixed by config sweeps on hardware)
N3 = 7                   # 3x3 channels offloaded to the Vector engine
TAP_MODE = "vs"          # conv taps as tensor_scalar_mul + tensor_add, all DVE
DSPLIT = 0.0             # PSUM banks per channel drained on DVE (rest on ACT)
OUT_Q = "sp"             # DMA queue for outputs: "act" (scalar HWDGE) or "sp"
PE_PACK = "skip"         # "skip": 3D rhs AP skips gap cols (F=448, dense PSUM);
                         # "stream": 2D rhs streams gaps (F=4*rw)
PARITY_TAPS = True       # even-offset taps first (x_o DMA cover)
TAIL_MODE = False        # True: last PE channel drains per-bank — shorter
                         # one-shot tail but ~4us slower in steady state
                         # (per-bank y DMAs + lhsT reload every matmul)
XPOOL_BUFS = 5           # x staging depth (PE) — 5/3 beat 4/2 by ~2.4us
DPOOL_BUFS = 3           # x staging depth (DVE)   (smoother DMA pipeline)

MM_MODE = f"bf16 n3={N3} {TAP_MODE}"  # informational (test.py prints it)

# Per-core channel order: interleave 7,5,3 so early big-k channels cover the
# DMA prefetch of later ones.
CORE_KS = [7, 5, 3] * CH_PER_GROUP_PER_CORE
DVE_POS = [i for i, k in enumerate(CORE_KS) if k == 3][:N3]
PE_POS = [i for i in range(CH_PER_CORE) if i not in DVE_POS]
KS_PE = [CORE_KS[i] for i in PE_POS]
TOFF = np.cumsum([0] + KS_PE).tolist()    # tmat slice offset per PE channel
N_TMAT = TOFF[-1]
N_PE = len(PE_POS)

_BASS_CACHE = {}


def _build_bass(reps=1):
    bf = mybir.dt.bfloat16
    nc = bacc.Bacc("TRN2", target_bir_lowering=False, debug=False)
    xp_d = nc.dram_tensor("xp", [N_PE, H, XCOLS], bf, kind="ExternalInput")
    t_d = nc.dram_tensor("tmat", [N_TMAT * H, H], bf, kind="ExternalInput")
    y_d = nc.dram_tensor("y", [N_PE, H, OCOLS], bf, kind="ExternalOutput")
    if N3:
        xd_d = nc.dram_tensor("xd", [N3, 128, XDF], bf, kind="ExternalInput")
        w_d = nc.dram_tensor("wdve", [128, N3 * 9], mybir.dt.float32, kind="ExternalInput")
        y2_d = nc.dram_tensor("y2", [N3, 128, ODF], bf, kind="ExternalOutput")

    with tile.TileContext(nc) as tc:
        with (
            tc.tile_pool(name="xpool", bufs=XPOOL_BUFS) as xpool,
            tc.tile_pool(name="tpool", bufs=1) as tpool,
            tc.tile_pool(name="opool", bufs=3) as opool,
            tc.tile_pool(name="dpool", bufs=DPOOL_BUFS) as dpool,
            tc.tile_pool(name="spool", bufs=3) as spool,
            tc.tile_pool(name="pspool", bufs=8, space="PSUM") as pspool,
        ):
            # Toeplitz bank resident in SBUF, one per-channel slice DMA each.
            t_t = tpool.tile([H, N_TMAT * H], bf, tag="t", name="tmat_sb")
            for ci in range(N_PE):
                k = KS_PE[ci]
                nc.sync.dma_start(
                    t_t[:, TOFF[ci] * H : (TOFF[ci] + k) * H],
                    t_d[TOFF[ci] * H : (TOFF[ci] + k) * H].rearrange(
                        "(p d) m -> p (d m)", d=k
                    ),
                )
            if N3:
                w_t = tpool.tile([128, N3 * 9], mybir.dt.float32, tag="w", name="wdve_sb")
                nc.sync.dma_start(w_t[:, :], w_d[:, :])

            def out_dma(dst, srcap):
                (nc.scalar if OUT_Q == "act" else nc.sync).dma_start(dst, srcap)

            def pe_channel(ci, tail=False):
                k = KS_PE[ci]
                rw, n_mm, xc = RW[k], N_MM_K[k], XCOLS_K[k]
                x_t = xpool.tile([H, XCOLS], bf, tag="x", name=f"x{ci}")
                if ci == 0:
                    # split the critical-path first DMA so pass 0 can start
                    # after half the transfer has landed
                    nc.sync.dma_start(x_t[:, : xc // 2], xp_d[ci][:, : xc // 2])
                    nc.sync.dma_start(x_t[:, xc // 2 : xc], xp_d[ci][:, xc // 2 : xc])
                else:
                    nc.sync.dma_start(x_t[:, :xc], xp_d[ci][:, :xc])
                out_t = opool.tile([H, OCOLS], bf, tag="o", name=f"o{ci}")
                # 8 single-bank PSUM tiles: cross-bank PSUM reads cost a flat
                # ~2.9us, single-bank DVE drains ~0.5us.
                pts = [
                    pspool.tile([H, N_MM_K[7]], mybir.dt.float32, tag="ps",
                                name=f"ps{ci}_{b}")
                    for b in range(N_CHUNK)
                ]

                def mm(dx, b):
                    lhsT = t_t[:, (TOFF[ci] + dx) * H : (TOFF[ci] + dx + 1) * H]
                    base = 4 * b * rw
                    if PE_PACK == "skip":
                        # 3D moving AP streams only the 4x112 data cols,
                        # skipping the pad gaps -> F=448, dense PSUM
                        nc.tensor.matmul(
                            pts[b][:, : 4 * W],
                            lhsT=lhsT,
                            rhs=x_t[:, base + dx : base + dx + n_mm]
                            .rearrange("p (i g) -> p i g", i=4)[:, :, :W],
                            start=(dx == 0),
                            stop=(dx == k - 1),
                        )
                    else:
                        nc.tensor.matmul(
                            pts[b][:, :n_mm],
                            lhsT=lhsT,
                            rhs=x_t[:, base + dx : base + dx + n_mm],
                            start=(dx == 0),
                            stop=(dx == k - 1),
                        )

                ov = out_t.rearrange("p (i w) -> p i w", i=N_IMGS)

                def drain(b):
                    img0 = 4 * b
                    if PE_PACK == "skip":
                        src = pts[b][:, : 4 * W]   # dense: fully-2D drain
                        dst = out_t[:, img0 * W : (img0 + 4) * W]
                    else:
                        src = pts[b][:, :n_mm].rearrange(
                            "p (i r) -> p i r", i=4)[:, :, :W]
                        dst = ov[:, img0 : img0 + 4, :]
                    if b < int(DSPLIT) + (1 if (DSPLIT % 1) and ci % 2 else 0):
                        nc.vector.tensor_copy(out=dst, in_=src)
                    else:
                        nc.scalar.copy(dst, src)
                    if tail:
                        # tail channel: DMA each 4-image piece right away
                        out_dma(y_d[ci][:, img0 * W : (img0 + 4) * W],
                                out_t[:, img0 * W : (img0 + 4) * W])
                    elif b == N_CHUNK // 2 - 1:
                        out_dma(y_d[ci][:, : OCOLS // 2],
                                out_t[:, : OCOLS // 2])

                if tail:
                    # last channel: finish each bank then drain+DMA it, so the
                    # drain/DMA tail overlaps this channel's own matmuls
                    # (lhsT reloads every MM, but only k*8 small MMs here)
                    for b in range(N_CHUNK):
                        for dx in range(k):
                            mm(dx, b)
                        drain(b)
                else:
                    # pass-major: one lhsT load per dx, 8 banks stream under it
                    for dx in range(k):
                        for b in range(N_CHUNK):
                            mm(dx, b)
                    for b in range(N_CHUNK):
                        drain(b)
                    out_dma(y_d[ci][:, OCOLS // 2 :], out_t[:, OCOLS // 2 :])

            def dve_channel(di):
                x_e = dpool.tile([128, XDF], bf, tag="xe", name=f"xe{di}")
                nc.sync.dma_start(x_e[:, :], xd_d[di])
                x_o = dpool.tile([128, XDF], bf, tag="xo", name=f"xo{di}")
                nc.sync.dma_start(x_o[:, : XDF - 1], xd_d[di][:, 1:XDF])
                accs = [
                    spool.tile([128, ODF], bf, tag=f"a{j}", name=f"acc{j}_{di}")
                    for j in range(2)
                ]

                def tap_ap(dy, dx):
                    off = (1 + dy) * RWP + (1 + dx)
                    src, o = (x_e, off) if off % 2 == 0 else (x_o, off - 1)
                    return src[:, o : o + HB * RWP].rearrange(
                        "p (r c) -> p r c", r=HB
                    )[:, :, :W]

                def wap(t):
                    return w_t[:, di * 9 + t : di * 9 + t + 1]

                def scale_to(dst, t):
                    dy, dx = TAPS3[t]
                    # "mix": one scale per channel on DVE, rest on ACT
                    # "vs1": only the last tap's mul on ACT (small DVE relief)
                    on_act = (TAP_MODE == "as" or (TAP_MODE == "mix" and t != 0)
                              or (TAP_MODE == "vs1" and t == 8))
                    if on_act:
                        nc.scalar.activation(
                            dst, tap_ap(dy, dx),
                            mybir.ActivationFunctionType.Copy, scale=wap(t),
                        )
                    else:
                        nc.vector.tensor_scalar_mul(dst, tap_ap(dy, dx), wap(t))

                a3 = [a.rearrange("p (r c) -> p r c", r=HB) for a in accs]
                # even-offset taps (dx=+-1) first: they read x_e only, giving
                # the x_o DMA ~15us of cover before any odd tap needs it
                if PARITY_TAPS:
                    order = sorted(range(9), key=lambda t: (1 + TAPS3[t][1]) % 2)
                else:
                    order = list(range(9))
                scale_to(a3[0], order[0])
                cur = 0
                for pos, t in enumerate(order[1:], start=1):
                    nxt = 1 - cur
                    if TAP_MODE == "stt":
                        dy, dx = TAPS3[t]
                        nc.vector.scalar_tensor_tensor(
                            out=a3[nxt], in0=tap_ap(dy, dx), scalar=wap(t),
                            in1=a3[cur],
                            op0=mybir.AluOpType.mult, op1=mybir.AluOpType.add,
                        )
                    else:
                        s_t = spool.tile([128, ODF], bf, tag="s",
                                         name=f"s{di}_{t}")
                        s3 = s_t.rearrange("p (r c) -> p r c", r=HB)
                        scale_to(s3, t)
                        # "mix": spill 3 of the 8 adds per channel to GPSIMD
                        # "vsg2": spill 2 (Pool idle since drains moved to ACT)
                        spill = ((TAP_MODE == "mix" and t in (2, 5, 8))
                                 or (TAP_MODE == "vsg2" and pos in (3, 6)))
                        if spill:
                            nc.gpsimd.tensor_add(a3[nxt], a3[cur], s3)
                        else:
                            nc.vector.tensor_add(a3[nxt], a3[cur], s3)
                    cur = nxt
                out_dma(y2_d[di], accs[cur][:, :])

            def body():
                pe_i = dve_i = 0
                for pos in range(CH_PER_CORE):
                    if pos in DVE_POS:
                        dve_channel(dve_i)
                        dve_i += 1
                    else:
                        pe_channel(pe_i, tail=TAIL_MODE and pe_i == N_PE - 1)
                        pe_i += 1

            if reps == 1:
                body()
            else:
                with tc.For_i(0, reps, 1):
                    body()
    nc.compile()
    return nc


def _get_bass(reps=1):
    if reps not in _BASS_CACHE:
        _BASS_CACHE[reps] = _build_bass(reps)
    return _BASS_CACHE[reps]


def _build_toeplitz(w, k):
    """w: [C, 1, k, k] -> T: [C, k, H, H], T[c,dx,hin,hout] = w[c,0,hin-hout+pad,dx]."""
    pad = (k - 1) // 2
    C = w.shape[0]
    T = np.zeros((C, k, H, H), np.float32)
    for dy in range(k):
        off = pad - dy  # hout = hin + off
        hin = np.arange(max(0, -off), H - max(0, off))
        T[:, :, hin, hin + off] = w[:, 0, dy, :][:, :, None]
    return T


def _core_channels(core):
    """Global channel ids in this core's processing order (7,5,3 interleave)."""
    out = []
    for j in range(CH_PER_GROUP_PER_CORE):
        for g in (2, 1, 0):  # k=7, 5, 3 groups
            out.append(g * GROUP_SIZE + core * CH_PER_GROUP_PER_CORE + j)
    return out


def _prepare_in_maps(x, w3, w5, w7):
    x = np.asarray(x, dtype=np.float32).astype(BF16)
    ws = {3: np.asarray(w3, np.float32), 5: np.asarray(w5, np.float32),
          7: np.asarray(w7, np.float32)}
    Ts = {k: _build_toeplitz(ws[k], k) for k in (5, 7) if True}
    Ts[3] = _build_toeplitz(ws[3], 3)

    in_maps = []
    for core in range(N_CORES):
        chs = _core_channels(core)
        pe_chs = [chs[i] for i in PE_POS]
        dve_chs = [chs[i] for i in DVE_POS]

        # staged x (PE): [N_PE, H, XCOLS], data at [i*rw+pad, i*rw+pad+W)
        xp = np.zeros((N_PE, H, XCOLS), BF16)
        for ci, gch in enumerate(pe_chs):
            k = KS_PE[ci]
            rw, pad = RW[k], (k - 1) // 2
            xv = xp[ci, :, : N_IMGS * rw].reshape(H, N_IMGS, rw)
            xv[:, :, pad : pad + W] = x[:, gch].transpose(1, 0, 2)

        # tmat blocks: per PE channel [hin, dx, hout] -> [(hin dx), hout]
        blocks = []
        for ci, gch in enumerate(pe_chs):
            k = KS_PE[ci]
            Tc = Ts[k][gch % GROUP_SIZE]  # [dx, hin, hout]
            blocks.append(
                np.ascontiguousarray(Tc.transpose(1, 0, 2)).reshape(k * H, H)
            )
        tml = np.concatenate(blocks, axis=0)
        assert tml.shape[0] == N_TMAT * H
        m = {"xp": xp, "tmat": tml.astype(BF16)}

        if N3:
            # staged x (DVE): [N3, 128, XDF]; partition = img*4 + hblock,
            # free = halo'd 30x114 patch
            xd = np.zeros((N3, 128, XDF), BF16)
            xdv = xd[:, :, : RH * RWP].reshape(N3, N_IMGS, 4, RH, RWP)
            for di, gch in enumerate(dve_chs):
                pad_img = np.zeros((N_IMGS, H + 2, RWP), BF16)
                pad_img[:, 1 : H + 1, 1 : W + 1] = x[:, gch]
                for hb in range(4):
                    xdv[di, :, hb] = pad_img[:, HB * hb : HB * hb + RH, :]
            # tap weights broadcast across partitions: [128, N3*9]
            wd = np.zeros((N3, 9), np.float32)
            for di, gch in enumerate(dve_chs):
                wd[di] = ws[3][gch % GROUP_SIZE, 0].reshape(9)
            m["xd"] = xd
            m["wdve"] = np.ascontiguousarray(
                np.broadcast_to(wd.reshape(1, N3 * 9), (128, N3 * 9))
            )
        in_maps.append(m)
    return in_maps


def _gather(results):
    out = np.empty((N_IMGS, GROUP_SIZE * len(GROUP_KS), H, W), np.float32)
    for core in range(N_CORES):
        chs = _core_channels(core)
        pe_chs = [chs[i] for i in PE_POS]
        y = np.asarray(results[core]["y"]).astype(np.float32)
        y = y.reshape(N_PE, H, N_IMGS, W)
        out[:, pe_chs] = y.transpose(2, 0, 1, 3)
        if N3:
            dve_chs = [chs[i] for i in DVE_POS]
            y2 = np.asarray(results[core]["y2"]).astype(np.float32)
            y2 = y2.reshape(N3, N_IMGS, 4, HB, W)
            for di, gch in enumerate(dve_chs):
                out[:, gch] = y2[di].reshape(N_IMGS, H, W)
    return out


def run(x, w3, w5, w7, **spmd_kwargs):
    """Full run; returns (output, BassKernelResults) for profiling access."""
    nc = _get_bass()
    in_maps = _prepare_in_maps(x, w3, w5, w7)
    br = run_bass_kernel_spmd(nc, in_maps, core_ids=list(range(N_CORES)), **spmd_kwargs)
    return _gather(br.results), br


def kernel(x, w3, w5, w7):
    out, _ = run(x, w3, w5, w7)
    return out

